# revision 40
# baseline (speedup 1.0000x reference)
"""Trainium2 Bass kernel for nn_LunaCausalAttention.

Sharding: 8 cores; core c handles batch b = c//4 and heads hs = 4*(c%4) .. hs+4.

Restructured vs baseline:
- DMA ordered so the pc projection streams against the xt tiles (no startup
  bubble); per-projection psum chains get enough banks to pipeline.
- Pass-1 computed m-major (awT = Z^T tril(G) + S^T Q), with rc folded into a
  pre-scaled qTrc at projection time, so the softmax exp emerges directly in
  the [m, tok] layout pass-2 needs -- no P~ transposes on the critical path.
- Softmax normalization deferred: P~ left unnormalized; a per-(head, token)
  scale tile (built by tiny fp16 outer-product matmuls) is applied once when
  attn^T leaves psum.
- S/T state accumulated directly in a persistent psum bank by the PE
  (start=c==0), copied to sbuf bf16 once per chunk on the Act engine.
- Output projection bias moved to the host-side partial reduction.
"""
import numpy as np

import concourse.bass as bass
import concourse.mybir as mybir
import concourse.tile as tile
from concourse import bacc
from concourse.masks import make_upper_triangular, make_identity
from concourse.bass_utils import run_bass_kernel_spmd

# static shapes
B, N, D, M, H, DH = 2, 1024, 1024, 64, 16, 64
C = 128                 # token chunk
NCH = N // C            # 8 chunks
NCORES = 8
HPC = 4                 # heads per core
E = HPC * DH            # 256 per-core head features
NF = D // 128           # 8 contraction tiles
BETA = float(np.log(2.0))
SCALE = DH ** -0.5

F32 = mybir.dt.float32
F16 = mybir.dt.float16
BF16 = mybir.dt.bfloat16
ADT = BF16              # attention-core operand dtype
AF = mybir.ActivationFunctionType
ALU = mybir.AluOpType


def build_bass(phase=3):
    nc = bacc.Bacc(None, target_bir_lowering=False)

    # ---- I/O ----
    xT_d = nc.dram_tensor("xT", [D, N], BF16, kind="ExternalInput")       # query[b].T
    pT_d = nc.dram_tensor("pT", [D, M], BF16, kind="ExternalInput")       # p[b].T
    wq_d = nc.dram_tensor("wq", [D, E], BF16, kind="ExternalInput")       # scale folded
    wk_d = nc.dram_tensor("wk", [D, E], BF16, kind="ExternalInput")
    wv_d = nc.dram_tensor("wv", [D, E], BF16, kind="ExternalInput")
    wpc_d = nc.dram_tensor("wpc", [D, E], BF16, kind="ExternalInput")
    wpq_d = nc.dram_tensor("wpq", [D, E], BF16, kind="ExternalInput")     # scale folded
    wo_d = nc.dram_tensor("wo", [E, D], BF16, kind="ExternalInput")
    bq_d = nc.dram_tensor("bq", [128, 2], F32, kind="ExternalInput")      # [i,et]=b[128et+i]
    bk_d = nc.dram_tensor("bk", [128, 2], F32, kind="ExternalInput")
    bpc_d = nc.dram_tensor("bpc", [128, 2], F32, kind="ExternalInput")
    bpq_d = nc.dram_tensor("bpq", [128, 2], F32, kind="ExternalInput")
    bvr_d = nc.dram_tensor("bvr", [1, E], BF16, kind="ExternalInput")     # row form
    rcb_d = nc.dram_tensor("rcb", [128, N], F32, kind="ExternalInput")    # every row = rc
    ones_d = nc.dram_tensor("onesr", [1, 128], BF16, kind="ExternalInput")
    rcc_d = nc.dram_tensor("rcc", [128, NCH], F32, kind="ExternalInput")
    ones2_d = nc.dram_tensor("ones2", [128, 2], BF16, kind="ExternalInput")  # h indicator
    out_d = nc.dram_tensor("outp", [N, D], F32, kind="ExternalOutput")

    with tile.TileContext(nc) as tc:
        with (
            tc.tile_pool(name="singles", bufs=1) as singles,
            tc.tile_pool(name="work", bufs=5) as work,
            tc.tile_pool(name="obuf", bufs=4) as obuf,
            tc.tile_pool(name="psum", bufs=1, space="PSUM") as psum,
        ):
            # ---- constants (device-generated) ----
            triu2 = singles.tile([128, 2 * C], F32)     # two upper-tri copies
            make_upper_triangular(nc, triu2[:, 0:C], val=1.0, diag=True)
            make_upper_triangular(nc, triu2[:, C:2 * C], val=1.0, diag=True)
            identb = singles.tile([128, 128], ADT)
            make_identity(nc, identb)

            # ---- DMA, in compute-need order ----
            def load_w(name, dram):
                w = singles.tile([128, NF, E], BF16, name=name)
                nc.sync.dma_start(
                    out=w, in_=dram.rearrange("(f p) e -> p f e", p=128))
                return w

            def load_small(shape, dt, dram, name):
                t = singles.tile(shape, dt, name=name)
                nc.sync.dma_start(out=t, in_=dram[:, :])
                return t

            wpc_sb = singles.tile([128, NF, E], BF16, name="wpc_sb")
            wpc_r = wpc_d.rearrange("(f p) e -> p f e", p=128)
            nc.sync.dma_start(out=wpc_sb[:, 0:2, :], in_=wpc_r[:, 0:2, :])
            bpc_sb = load_small([128, 2], F32, bpc_d, "bpc_sb")
            xt_sb = []
            for f in range(NF):
                xt = singles.tile([128, N], BF16, name=f"xt{f}")
                nc.sync.dma_start(out=xt, in_=xT_d[f * 128:(f + 1) * 128, :])
                xt_sb.append(xt)
                if f == 0:
                    nc.sync.dma_start(out=wpc_sb[:, 2:NF, :],
                                      in_=wpc_r[:, 2:NF, :])
            wk_sb = load_w("wk_sb", wk_d)
            bk_sb = load_small([128, 2], F32, bk_d, "bk_sb")
            wq_sb = load_w("wq_sb", wq_d)
            bq_sb = load_small([128, 2], F32, bq_d, "bq_sb")
            rcb_sb = singles.tile([128, N], F32)
            nc.sync.dma_start(out=rcb_sb, in_=rcb_d[:, :])
            wpq_sb = load_w("wpq_sb", wpq_d)
            pT_sb = singles.tile([128, NF, M], BF16)
            nc.sync.dma_start(
                out=pT_sb, in_=pT_d.rearrange("(f p) m -> p f m", p=128))
            bpq_sb = load_small([128, 2], F32, bpq_d, "bpq_sb")
            wv_sb = load_w("wv_sb", wv_d)
            bvr_sb = load_small([1, E], BF16, bvr_d, "bvr_sb")
            ones = load_small([1, 128], BF16, ones_d, "ones")
            rcc = load_small([128, NCH], F32, rcc_d, "rcc")
            ones2 = load_small([128, 2], BF16, ones2_d, "ones2")
            wo_sb = singles.tile([128, 2, D], BF16)
            nc.sync.dma_start(
                out=wo_sb, in_=wo_d.rearrange("(t p) o -> p t o", p=128))

            # ---- persistent sbuf tiles ----
            pcT_sb = singles.tile([128, 2, N], ADT)     # [feat, hp, tok]
            kT_sb = singles.tile([128, 2, N], ADT)
            qTrc_sb = singles.tile([128, 2, N], ADT)    # q * rc(tok), bias folded
            bdpq = singles.tile([128, 2, 128], ADT)     # block-diag pq per hp
            nc.vector.memset(bdpq, 0.0)
            vtok_sb = [singles.tile([128, E], ADT, name=f"vtok{t}")
                       for t in range(NCH)]
            attnT_sb = [singles.tile([128, 2, C], ADT, name=f"attnT{t}")
                        for t in range(NCH)]
            S_bd = [singles.tile([128, 128], ADT, name=f"S{hp}") for hp in range(2)]
            T_bd = [singles.tile([128, 128], ADT, name=f"T{hp}") for hp in range(2)]

            # persistent psum state bank, all block-diagonal [128,128] regions:
            #   S (feat-pair x m-pair): [:, 0:128] hp0, [:, 128:256] hp1
            #   T (m-pair x feat-pair): [:, 256:384] hp0, [:, 384:512] hp1
            state = psum.tile([128, 512], F32, tag="state", name="state")
            nc.vector.memset(state, 0.0)

            # ---- projections, emitted per token-half (nh) so attention on
            # chunks 0-3 can overlap the nh=1 projection work ----
            def proj_chain(kind, et, nh):
                w_sb, b_sb, dst = {"pc": (wpc_sb, bpc_sb, pcT_sb),
                                   "k": (wk_sb, bk_sb, kT_sb),
                                   "q": (wq_sb, bq_sb, qTrc_sb)}[kind]
                pp = psum.tile([128, 512], F32, tag="pp", bufs=2, name="ppc")
                for f in range(NF):
                    nc.tensor.matmul(
                        pp, w_sb[:, f, et * 128:(et + 1) * 128],
                        xt_sb[f][:, nh * 512:(nh + 1) * 512],
                        start=(f == 0), stop=(f == NF - 1))
                if kind == "q":
                    nc.vector.scalar_tensor_tensor(
                        dst[:, et, nh * 512:(nh + 1) * 512], pp,
                        b_sb[:, et:et + 1],
                        rcb_sb[:, nh * 512:(nh + 1) * 512],
                        ALU.add, ALU.mult)
                else:
                    nc.scalar.activation(
                        dst[:, et, nh * 512:(nh + 1) * 512], pp,
                        AF.Identity, bias=b_sb[:, et:et + 1])

            def proj_half(nh):
                for kind in ("pc", "k", "q"):
                    for et in range(2):
                        proj_chain(kind, et, nh)

            def proj_pq():
                for hp in range(2):
                    ppq = psum.tile([128, 512], F32, tag="pp", bufs=2,
                                    name="pppq")
                    for f in range(NF):
                        nc.tensor.matmul(
                            ppq[:, 0:M], wpq_sb[:, f, hp * 128:(hp + 1) * 128],
                            pT_sb[:, f, :],
                            start=(f == 0), stop=(f == NF - 1))
                    for h in range(2):
                        sl = slice(64 * h, 64 * h + 64)
                        nc.vector.tensor_scalar_add(
                            bdpq[sl, hp, 64 * h:64 * h + 64], ppq[sl, 0:M],
                            bpq_sb[sl, hp:hp + 1])

            def proj_v(tb):
                pkv = psum.tile([128, 512], F32, tag="pp", bufs=2, name="pkv")
                for f in range(NF):
                    nc.tensor.matmul(
                        pkv[:, 0:E], xt_sb[f][:, tb * 128:(tb + 1) * 128],
                        wv_sb[:, f, :],
                        start=(f == 0), stop=False)
                nc.tensor.matmul(pkv[:, 0:E], ones, bvr_sb,
                                 start=False, stop=True)
                nc.any.tensor_copy(vtok_sb[tb], pkv[:, 0:E])

            # ---- attention ----
            def attn_call(c, hp):
                tok = slice(c * C, (c + 1) * C)
                # psum packing
                A = psum.tile([128, 512], F32, tag="pca", bufs=2, name="A")
                Bp = psum.tile([128, 512], F32, tag="pcb", bufs=2, name="Bp")
                Cp = psum.tile([128, 512], F32, tag="pcc", bufs=1, name="Cp")
                pz = A[:, 0:128]
                awT = A[:, 128:256]
                gmp = (A[:, 256:384], Bp[:, 0:128])
                g2p = (A[:, 384:512], Bp[:, 128:256])
                pan = Bp[:, 256:384]
                pkt = Cp[:, 0:64].bitcast(ADT)
                att = Cp[:, 64:128].bitcast(ADT)
                rs = Cp[0:1, 128:384]
                ezp = Cp[:, 384:512]

                # Z_c: pz = pcT^T @ bdpq  -> [tok, m-pair]
                nc.tensor.matmul(pz, pcT_sb[:, hp, tok], bdpq[:, hp, :],
                                 start=True, stop=True)
                nc.scalar.activation(ezp, pz, AF.Exp, scale=BETA)
                z = work.tile([128, 128], ADT, name="z")
                nc.scalar.activation(z, ezp, AF.Ln, bias=1.0, scale=1.0)

                # K_tok via PE transpose
                ktc = work.tile([128, 128], ADT, name="ktc")
                nc.tensor.transpose(pkt, kT_sb[:, hp, tok], identb)
                nc.any.tensor_copy(ktc, pkt)

                # Z^T via PE transpose
                at = work.tile([128, 128], ADT, name="at")
                nc.tensor.transpose(att, z, identb)
                nc.any.tensor_copy(at, att)

                # G^T = K Q_rc^T (rc folded in qTrc), masked
                gm = work.tile([128, 256], ADT, name="gm")
                for h in range(2):
                    s = slice(64 * h, 64 * h + 64)
                    nc.tensor.matmul(gmp[h], kT_sb[s, hp, tok],
                                     qTrc_sb[s, hp, tok], start=True, stop=True,
                                     tile_position=(64 * h, 0))
                    nc.any.tensor_mul(gm[:, 128 * h:128 * h + 128],
                                      gmp[h], triu2[:, 0:C])

                # awT[m-pair, tok] = Z^T Gm (+ S^T Q_rc)
                for h in range(2):
                    s = slice(64 * h, 64 * h + 64)
                    nc.tensor.matmul(awT[s, :], z[:, s],
                                     gm[:, 128 * h:128 * h + 128],
                                     start=True, stop=(c == 0),
                                     tile_position=(0, 64 * h))
                if c > 0:
                    nc.tensor.matmul(awT, S_bd[hp], qTrc_sb[:, hp, tok],
                                     start=False, stop=True,
                                     skip_group_check=True)

                # P~^T = exp(awT), unnormalized, directly m-major
                pt = work.tile([128, 128], ADT, name="pt")
                nc.scalar.activation(pt, awT, AF.Exp, scale=1.0)

                # rowsums over m (partition dim) -> [1, 2*128] on partition 0
                for h in range(2):
                    nc.tensor.matmul(rs[:, 128 * h:128 * h + 128],
                                     ones2[:, h:h + 1], pt,
                                     start=True, stop=True,
                                     tile_position=(0, 0))
                rcp = work.tile([1, 256], F32, name="rcp")
                nc.vector.reciprocal(rcp, rs)
                if phase == 2:
                    rcf2 = work.tile([128, 256], F32, name="rcf2")
                    nc.vector.memset(rcf2, 0.0)
                    nc.vector.tensor_copy(rcf2[0:1, :], rcp)
                else:
                    rcf2 = None
                # layered broadcast of 1/rowsum on Pool; rc folds into outproj
                scl = work.tile([128, 128], F32, name="scl")
                nc.gpsimd.partition_broadcast(scl, rcp[:, 128:256], channels=128)
                nc.gpsimd.partition_broadcast(
                    scl[0:64, :], rcp[:, 0:128], channels=64)

                # pass 2: G2^T = Z P~^T, masked
                g2m = work.tile([128, 256], ADT, name="g2m")
                for h in range(2):
                    s = slice(64 * h, 64 * h + 64)
                    nc.tensor.matmul(g2p[h], at[s, :], pt[s, :],
                                     start=True, stop=True,
                                     tile_position=(64 * h, 0))
                    nc.any.tensor_mul(g2m[:, 128 * h:128 * h + 128],
                                      g2p[h], triu2[:, 0:C])

                # attn^T = V^T G2m (+ T^T P~^T), then normalize via scl
                for h in range(2):
                    nc.tensor.matmul(
                        pan[64 * h:64 * h + 64, :],
                        vtok_sb[c][:, hp * 128 + 64 * h:hp * 128 + 64 * h + 64],
                        g2m[:, 128 * h:128 * h + 128],
                        start=True, stop=(c == 0),
                        tile_position=(0, 64 * h))
                if c > 0:
                    nc.tensor.matmul(pan, T_bd[hp], pt,
                                     start=False, stop=True,
                                     skip_group_check=True)
                nc.any.tensor_mul(attnT_sb[c][:, hp, :], pan, scl)

                # ---- state updates (block-diag accumulate in psum) ----
                Sp = state[:, 256 + 128 * hp:384 + 128 * hp]
                Tp = state[:, 128 * hp:128 * hp + 128]
                for h in range(2):
                    s = slice(64 * h, 64 * h + 64)
                    nc.tensor.matmul(Sp[s, s], ktc[:, s], z[:, s],
                                     start=False, stop=True,
                                     skip_group_check=True,
                                     tile_position=(0, 64 * h))
                    nc.tensor.matmul(
                        Tp[s, s], z[:, s],
                        vtok_sb[c][:, hp * 128 + 64 * h:hp * 128 + 64 * h + 64],
                        start=False, stop=True,
                        skip_group_check=True,
                        tile_position=(0, 64 * h))
                if c < NCH - 1 or phase == 2:
                    nc.any.tensor_copy(S_bd[hp], Sp)
                    nc.any.tensor_copy(T_bd[hp], Tp)
                if phase == 2:
                    Sc = work.tile([128, 128], F32, name="Sc")
                    nc.vector.tensor_copy(Sc, Sp)
                    Tc = work.tile([128, 128], F32, name="Tc")
                    nc.vector.tensor_copy(Tc, Tp)
                    return dict(z=z, gm=gm, pt=pt, rcf2=rcf2, scl=scl,
                                g2m=g2m, Sc=Sc, Tc=Tc)

            def out_block(c):
                tok = slice(c * C, (c + 1) * C)
                for oh in range(2):
                    po = psum.tile([128, 512], F32, tag="pp", bufs=2, name="po")
                    for et in range(2):
                        nc.tensor.matmul(
                            po, attnT_sb[c][:, et, :],
                            wo_sb[:, et, oh * 512:(oh + 1) * 512],
                            start=(et == 0), stop=(et == 1))
                    ob = obuf.tile([128, 512], F32, name="ob")
                    nc.any.tensor_scalar_mul(ob, po, rcc[:, c:c + 1])
                    nc.sync.dma_start(
                        out=out_d[tok, oh * 512:(oh + 1) * 512], in_=ob)

            if phase in (1, 2):
                proj_half(0)
                proj_half(1)
                proj_pq()
                for c in range(NCH):
                    proj_v(c)
            if phase == 1:
                # dump projections (bf16 bitcast to f32 cols)
                for et in range(2):
                    nc.sync.dma_start(out=out_d[et * 128:(et + 1) * 128, 0:512],
                                      in_=qTrc_sb[:, et, :].bitcast(F32))
                    nc.sync.dma_start(
                        out=out_d[256 + et * 128:256 + (et + 1) * 128, 0:512],
                        in_=kT_sb[:, et, :].bitcast(F32))
                    nc.sync.dma_start(
                        out=out_d[512 + et * 128:512 + (et + 1) * 128, 0:512],
                        in_=pcT_sb[:, et, :].bitcast(F32))
                    nc.sync.dma_start(
                        out=out_d[768 + et * 128:768 + (et + 1) * 128, 0:64],
                        in_=bdpq[:, et, :].bitcast(F32))
                nc.sync.dma_start(out=out_d[768:896, 128:256],
                                  in_=vtok_sb[0].bitcast(F32))

            if phase >= 3:
                proj_half(0)
                proj_pq()
                proj_v(0)
                filler = ([lambda tb=tb: proj_v(tb) for tb in range(1, 4)]
                          + [lambda et=et, k=k: proj_chain(k, et, 1)
                             for k in ("pc", "k", "q") for et in range(2)]
                          + [lambda tb=tb: proj_v(tb) for tb in range(4, NCH)])
                fi = 0
                for c in range(NCH):
                    for hp in range(2):
                        attn_call(c, hp)
                        # drip-feed remaining projection work between calls
                        for _ in range(2 if c < 4 else 0):
                            if fi < len(filler):
                                filler[fi]()
                                fi += 1
                    out_block(c)
                while fi < len(filler):
                    filler[fi]()
                    fi += 1
            elif phase == 2:
                # run chunks 0 and 1 for hp=0 only, dump intermediates
                t0 = attn_call(0, 0)
                t1 = attn_call(1, 0)
                for i, t in enumerate((t0, t1)):
                    base = i * 512
                    nc.sync.dma_start(out=out_d[base + 0:base + 128, 0:64],
                                      in_=t["z"].bitcast(F32))
                    nc.sync.dma_start(out=out_d[base + 0:base + 128, 64:192],
                                      in_=t["gm"].bitcast(F32))
                    nc.sync.dma_start(out=out_d[base + 0:base + 128, 192:256],
                                      in_=t["pt"].bitcast(F32))
                    nc.sync.dma_start(out=out_d[base + 128:base + 256, 0:256],
                                      in_=t["rcf2"])
                    nc.sync.dma_start(out=out_d[base + 128:base + 256, 256:512],
                                      in_=t["scl"])
                    nc.sync.dma_start(out=out_d[base + 256:base + 384, 0:128],
                                      in_=t["g2m"].bitcast(F32))
                    nc.sync.dma_start(out=out_d[base + 256:base + 384, 128:256],
                                      in_=t["Sc"])
                    nc.sync.dma_start(out=out_d[base + 256:base + 384, 256:384],
                                      in_=t["Tc"])
                    nc.sync.dma_start(out=out_d[base + 384:base + 512, 0:64],
                                      in_=attnT_sb[i][:, 0, :].bitcast(F32))

    # Patch the act-table map so Exp and Ln both resolve to the combined
    # natural_log_exp_and_others set (otherwise the load-placement pass
    # alternates exp_and_others <-> natural_log per chunk, ~42us of reloads).
    import concourse.bacc as _bacc_mod
    from concourse.hw_specs import get_activation_tables as _gat
    _orig_gat = _bacc_mod.get_activation_tables

    def _patched_gat(arch):
        t = _gat(arch)
        for name, s in t.items():
            if name != "natural_log_exp_and_others":
                s.discard(AF.Exp)
                s.discard(AF.Ln)
        return t

    _bacc_mod.get_activation_tables = _patched_gat
    try:
        nc.compile()
    finally:
        _bacc_mod.get_activation_tables = _orig_gat
    return nc


_CACHE = {}


def _get_nc():
    import os
    phase = int(os.environ.get("KPHASE", "3"))
    key = f"nc{phase}"
    if key not in _CACHE:
        _CACHE[key] = build_bass(phase)
    return _CACHE[key]


def make_in_maps(query, p, Wq, bq, Wpq, bpq, Wpc, bpc, Wk, bk, Wv, bv, Wo, bo):
    import ml_dtypes
    bf = ml_dtypes.bfloat16
    f32 = lambda a: np.ascontiguousarray(np.asarray(a), dtype=np.float32)
    query, p = f32(query), f32(p)
    Wq, Wpq, Wpc, Wk, Wv, Wo = map(f32, (Wq, Wpq, Wpc, Wk, Wv, Wo))
    bq, bpq, bpc, bk, bv, bo = map(f32, (bq, bpq, bpc, bk, bv, bo))
    rc = (1.0 / ((np.arange(N) + 1.0) * BETA)).astype(np.float32)
    rcb = np.ascontiguousarray(np.broadcast_to(rc[None, :], (128, N)))
    ones2 = np.zeros((128, 2), bf)
    ones2[0:64, 0] = 1
    ones2[64:128, 1] = 1
    rcc = np.ascontiguousarray(rc.reshape(NCH, 128).T)

    def col2(v):  # (256,) -> (128, 2)
        return np.ascontiguousarray(v.reshape(2, 128).T)

    in_maps = []
    for core in range(NCORES):
        b = core // 4
        hs = (core % 4) * HPC
        cols = slice(hs * DH, (hs + HPC) * DH)
        m = {
            "xT": np.ascontiguousarray(query[b].T).astype(bf),
            "pT": np.ascontiguousarray(p[b].T).astype(bf),
            "wq": np.ascontiguousarray((Wq[cols, :] * SCALE).T).astype(bf),
            "wk": np.ascontiguousarray(Wk[cols, :].T).astype(bf),
            "wv": np.ascontiguousarray(Wv[cols, :].T).astype(bf),
            "wpc": np.ascontiguousarray(Wpc[cols, :].T).astype(bf),
            "wpq": np.ascontiguousarray((Wpq[cols, :] * SCALE).T).astype(bf),
            "wo": np.ascontiguousarray(Wo[:, cols].T).astype(bf),
            "bq": col2(bq[cols] * SCALE),
            "bk": col2(bk[cols]),
            "bpc": col2(bpc[cols]),
            "bpq": col2(bpq[cols] * SCALE),
            "bvr": np.ascontiguousarray(bv[cols].reshape(1, E)).astype(bf),
            "rcb": rcb,
            "onesr": np.ones((1, 128), bf),
            "rcc": rcc,
            "ones2": ones2,
        }
        in_maps.append(m)
    return in_maps


def kernel(query, p, dec_input_mask=None, p_mask=None,
           Wq=None, bq=None, Wpq=None, bpq=None, Wpc=None, bpc=None,
           Wk=None, bk=None, Wv=None, bv=None, Wo=None, bo=None,
           _trace=False, _trace_kwargs=None):
    in_maps = make_in_maps(query, p, Wq, bq, Wpq, bpq, Wpc, bpc,
                           Wk, bk, Wv, bv, Wo, bo)
    res = run_bass_kernel_spmd(_get_nc(), in_maps, core_ids=list(range(NCORES)),
                               trace=_trace, **(_trace_kwargs or {}))
    bo = np.asarray(bo, dtype=np.float32)
    out = np.zeros((B, N, D), np.float32)
    out += bo.reshape(1, 1, D)
    for core in range(NCORES):
        out[core // 4] += res.results[core]["outp"]
    if _trace:
        kernel.last_result = res
    return out


# revision 41
# speedup vs baseline: 1.1346x; 1.1346x over previous
"""Trainium2 Bass kernel for nn_LunaCausalAttention.

Sharding: 8 cores; core c handles batch b = c//4 and heads hs = 4*(c%4) .. hs+4.

Restructured vs baseline:
- DMA ordered so the pc projection streams against the xt tiles (no startup
  bubble); per-projection psum chains get enough banks to pipeline.
- Pass-1 computed m-major (awT = Z^T tril(G) + S^T Q), with rc folded into a
  pre-scaled qTrc at projection time, so the softmax exp emerges directly in
  the [m, tok] layout pass-2 needs -- no P~ transposes on the critical path.
- Softmax normalization deferred: P~ left unnormalized; a per-(head, token)
  scale tile (built by tiny fp16 outer-product matmuls) is applied once when
  attn^T leaves psum.
- S/T state accumulated directly in a persistent psum bank by the PE
  (start=c==0), copied to sbuf bf16 once per chunk on the Act engine.
- Output projection bias moved to the host-side partial reduction.
"""
import numpy as np

import concourse.bass as bass
import concourse.mybir as mybir
import concourse.tile as tile
from concourse import bacc
from concourse.masks import make_upper_triangular, make_identity
from concourse.bass_utils import run_bass_kernel_spmd

# static shapes
B, N, D, M, H, DH = 2, 1024, 1024, 64, 16, 64
C = 128                 # token chunk
NCH = N // C            # 8 chunks
NCORES = 8
HPC = 4                 # heads per core
E = HPC * DH            # 256 per-core head features
NF = D // 128           # 8 contraction tiles
BETA = float(np.log(2.0))
SCALE = DH ** -0.5

F32 = mybir.dt.float32
F16 = mybir.dt.float16
BF16 = mybir.dt.bfloat16
ADT = BF16              # attention-core operand dtype
AF = mybir.ActivationFunctionType
ALU = mybir.AluOpType


def build_bass(phase=3):
    nc = bacc.Bacc(None, target_bir_lowering=False)

    # ---- I/O ----
    xT_d = nc.dram_tensor("xT", [D, N], BF16, kind="ExternalInput")       # query[b].T
    pT_d = nc.dram_tensor("pT", [D, M], BF16, kind="ExternalInput")       # p[b].T
    wq_d = nc.dram_tensor("wq", [D, E], BF16, kind="ExternalInput")       # scale folded
    wk_d = nc.dram_tensor("wk", [D, E], BF16, kind="ExternalInput")
    wv_d = nc.dram_tensor("wv", [D, E], BF16, kind="ExternalInput")
    wpc_d = nc.dram_tensor("wpc", [D, E], BF16, kind="ExternalInput")
    wpq_d = nc.dram_tensor("wpq", [D, E], BF16, kind="ExternalInput")     # scale folded
    wo_d = nc.dram_tensor("wo", [E, D], BF16, kind="ExternalInput")
    bq_d = nc.dram_tensor("bq", [128, 2], F32, kind="ExternalInput")      # [i,et]=b[128et+i]
    bk_d = nc.dram_tensor("bk", [128, 2], F32, kind="ExternalInput")
    bpc_d = nc.dram_tensor("bpc", [128, 2], F32, kind="ExternalInput")
    bpq_d = nc.dram_tensor("bpq", [128, 2], F32, kind="ExternalInput")
    bvr_d = nc.dram_tensor("bvr", [1, E], BF16, kind="ExternalInput")     # row form
    rcb_d = nc.dram_tensor("rcb", [128, N], F32, kind="ExternalInput")    # every row = rc
    ones_d = nc.dram_tensor("onesr", [1, 128], BF16, kind="ExternalInput")
    rcc_d = nc.dram_tensor("rcc", [128, NCH], F32, kind="ExternalInput")
    ones2_d = nc.dram_tensor("ones2", [128, 2], BF16, kind="ExternalInput")  # h indicator
    out_d = nc.dram_tensor("outp", [N, D], F32, kind="ExternalOutput")

    with tile.TileContext(nc) as tc:
        with (
            tc.tile_pool(name="singles", bufs=1) as singles,
            tc.tile_pool(name="work", bufs=5) as work,
            tc.tile_pool(name="obuf", bufs=4) as obuf,
            tc.tile_pool(name="psum", bufs=1, space="PSUM") as psum,
        ):
            # ---- constants (device-generated) ----
            triu2 = singles.tile([128, 2 * C], F32)     # two upper-tri copies
            make_upper_triangular(nc, triu2[:, 0:C], val=1.0, diag=True)
            make_upper_triangular(nc, triu2[:, C:2 * C], val=1.0, diag=True)
            identb = singles.tile([128, 128], ADT)
            make_identity(nc, identb)

            # ---- DMA, in compute-need order ----
            def load_w(name, dram):
                w = singles.tile([128, NF, E], BF16, name=name)
                nc.sync.dma_start(
                    out=w, in_=dram.rearrange("(f p) e -> p f e", p=128))
                return w

            def load_small(shape, dt, dram, name):
                t = singles.tile(shape, dt, name=name)
                nc.sync.dma_start(out=t, in_=dram[:, :])
                return t

            wpc_sb = singles.tile([128, NF, E], BF16, name="wpc_sb")
            wpc_r = wpc_d.rearrange("(f p) e -> p f e", p=128)
            nc.sync.dma_start(out=wpc_sb[:, 0:2, :], in_=wpc_r[:, 0:2, :])
            bpc_sb = load_small([128, 2], F32, bpc_d, "bpc_sb")
            xt_sb = []
            for f in range(NF):
                xt = singles.tile([128, N], BF16, name=f"xt{f}")
                nc.sync.dma_start(out=xt, in_=xT_d[f * 128:(f + 1) * 128, :])
                xt_sb.append(xt)
                if f == 0:
                    nc.sync.dma_start(out=wpc_sb[:, 2:NF, :],
                                      in_=wpc_r[:, 2:NF, :])
            wk_sb = load_w("wk_sb", wk_d)
            bk_sb = load_small([128, 2], F32, bk_d, "bk_sb")
            wq_sb = load_w("wq_sb", wq_d)
            bq_sb = load_small([128, 2], F32, bq_d, "bq_sb")
            rcb_sb = singles.tile([128, N], F32)
            nc.sync.dma_start(out=rcb_sb, in_=rcb_d[:, :])
            wpq_sb = load_w("wpq_sb", wpq_d)
            pT_sb = singles.tile([128, NF, M], BF16)
            nc.sync.dma_start(
                out=pT_sb, in_=pT_d.rearrange("(f p) m -> p f m", p=128))
            bpq_sb = load_small([128, 2], F32, bpq_d, "bpq_sb")
            wv_sb = load_w("wv_sb", wv_d)
            bvr_sb = load_small([1, E], BF16, bvr_d, "bvr_sb")
            ones = load_small([1, 128], BF16, ones_d, "ones")
            rcc = load_small([128, NCH], F32, rcc_d, "rcc")
            ones2 = load_small([128, 2], BF16, ones2_d, "ones2")
            wo_sb = singles.tile([128, 2, D], BF16)
            nc.sync.dma_start(
                out=wo_sb, in_=wo_d.rearrange("(t p) o -> p t o", p=128))

            # ---- persistent sbuf tiles ----
            pcT_sb = singles.tile([128, 2, N], ADT)     # [feat, hp, tok]
            kT_sb = singles.tile([128, 2, N], ADT)
            qTrc_sb = singles.tile([128, 2, N], ADT)    # q * rc(tok), bias folded
            bdpq = singles.tile([128, 2, 128], ADT)     # block-diag pq per hp
            nc.vector.memset(bdpq, 0.0)
            vtok_sb = [singles.tile([128, E], ADT, name=f"vtok{t}")
                       for t in range(NCH)]
            attnT_sb = [singles.tile([128, 2, C], ADT, name=f"attnT{t}")
                        for t in range(NCH)]
            S_bd = [singles.tile([128, 128], ADT, name=f"S{hp}") for hp in range(2)]
            T_bd = [singles.tile([128, 128], ADT, name=f"T{hp}") for hp in range(2)]

            # persistent psum state bank, all block-diagonal [128,128] regions:
            #   S (feat-pair x m-pair): [:, 0:128] hp0, [:, 128:256] hp1
            #   T (m-pair x feat-pair): [:, 256:384] hp0, [:, 384:512] hp1
            state = psum.tile([128, 512], F32, tag="state", name="state")
            nc.vector.memset(state, 0.0)

            # ---- projections, emitted per token-half (nh) so attention on
            # chunks 0-3 can overlap the nh=1 projection work ----
            def proj_chain(kind, et, nh):
                w_sb, b_sb, dst = {"pc": (wpc_sb, bpc_sb, pcT_sb),
                                   "k": (wk_sb, bk_sb, kT_sb),
                                   "q": (wq_sb, bq_sb, qTrc_sb)}[kind]
                pp = psum.tile([128, 512], F32, tag="pp", bufs=2, name="ppc")
                for f in range(NF):
                    nc.tensor.matmul(
                        pp, w_sb[:, f, et * 128:(et + 1) * 128],
                        xt_sb[f][:, nh * 512:(nh + 1) * 512],
                        start=(f == 0), stop=(f == NF - 1))
                if kind == "q":
                    nc.vector.scalar_tensor_tensor(
                        dst[:, et, nh * 512:(nh + 1) * 512], pp,
                        b_sb[:, et:et + 1],
                        rcb_sb[:, nh * 512:(nh + 1) * 512],
                        ALU.add, ALU.mult)
                else:
                    nc.scalar.activation(
                        dst[:, et, nh * 512:(nh + 1) * 512], pp,
                        AF.Identity, bias=b_sb[:, et:et + 1])

            def proj_half(nh):
                for kind in ("pc", "k", "q"):
                    for et in range(2):
                        proj_chain(kind, et, nh)

            def proj_pq():
                for hp in range(2):
                    ppq = psum.tile([128, 512], F32, tag="pp", bufs=2,
                                    name="pppq")
                    for f in range(NF):
                        nc.tensor.matmul(
                            ppq[:, 0:M], wpq_sb[:, f, hp * 128:(hp + 1) * 128],
                            pT_sb[:, f, :],
                            start=(f == 0), stop=(f == NF - 1))
                    for h in range(2):
                        sl = slice(64 * h, 64 * h + 64)
                        nc.vector.tensor_scalar_add(
                            bdpq[sl, hp, 64 * h:64 * h + 64], ppq[sl, 0:M],
                            bpq_sb[sl, hp:hp + 1])

            def proj_v(tb):
                pkv = psum.tile([128, 512], F32, tag="pp", bufs=2, name="pkv")
                for f in range(NF):
                    nc.tensor.matmul(
                        pkv[:, 0:E], xt_sb[f][:, tb * 128:(tb + 1) * 128],
                        wv_sb[:, f, :],
                        start=(f == 0), stop=False)
                nc.tensor.matmul(pkv[:, 0:E], ones, bvr_sb,
                                 start=False, stop=True)
                nc.any.tensor_copy(vtok_sb[tb], pkv[:, 0:E])

            # ---- attention ----
            def attn_call(c, hp):
                tok = slice(c * C, (c + 1) * C)
                # psum packing
                A = psum.tile([128, 512], F32, tag="pca", bufs=2, name="A")
                Bp = psum.tile([128, 512], F32, tag="pcb", bufs=2, name="Bp")
                Cp = psum.tile([128, 512], F32, tag="pcc", bufs=1, name="Cp")
                pz = A[:, 0:128]
                awT = A[:, 128:256]
                gmp = (A[:, 256:384], Bp[:, 0:128])
                g2p = (A[:, 384:512], Bp[:, 128:256])
                pan = Bp[:, 256:384]
                pkt = Cp[:, 0:64].bitcast(ADT)
                att = Cp[:, 64:128].bitcast(ADT)
                rs = Cp[0:1, 128:384]

                # Z_c: pz = pcT^T @ bdpq  -> [tok, m-pair]
                nc.tensor.matmul(pz, pcT_sb[:, hp, tok], bdpq[:, hp, :],
                                 start=True, stop=True)
                ez = work.tile([128, 128], F32, name="ez")
                nc.scalar.activation(ez, pz, AF.Exp, scale=BETA)
                z = work.tile([128, 128], ADT, name="z")
                nc.scalar.activation(z, ez, AF.Ln, bias=1.0, scale=1.0)

                # K_tok via PE transpose
                ktc = work.tile([128, 128], ADT, name="ktc")
                nc.tensor.transpose(pkt, kT_sb[:, hp, tok], identb)
                nc.any.tensor_copy(ktc, pkt)

                # Z^T via PE transpose
                at = work.tile([128, 128], ADT, name="at")
                nc.tensor.transpose(att, z, identb)
                nc.any.tensor_copy(at, att)

                # G^T = K Q_rc^T (rc folded in qTrc), masked
                gm = work.tile([128, 256], ADT, name="gm")
                for h in range(2):
                    s = slice(64 * h, 64 * h + 64)
                    nc.tensor.matmul(gmp[h], kT_sb[s, hp, tok],
                                     qTrc_sb[s, hp, tok], start=True, stop=True,
                                     tile_position=(64 * h, 0))
                    nc.any.tensor_mul(gm[:, 128 * h:128 * h + 128],
                                      gmp[h], triu2[:, 0:C])

                # awT[m-pair, tok] = Z^T Gm (+ S^T Q_rc)
                for h in range(2):
                    s = slice(64 * h, 64 * h + 64)
                    nc.tensor.matmul(awT[s, :], z[:, s],
                                     gm[:, 128 * h:128 * h + 128],
                                     start=True, stop=(c == 0),
                                     tile_position=(0, 64 * h))
                if c > 0:
                    nc.tensor.matmul(awT, S_bd[hp], qTrc_sb[:, hp, tok],
                                     start=False, stop=True,
                                     skip_group_check=True)

                # P~^T = exp(awT), unnormalized, directly m-major
                pt = work.tile([128, 128], ADT, name="pt")
                nc.scalar.activation(pt, awT, AF.Exp, scale=1.0)

                # rowsums over m (partition dim) -> [1, 2*128] on partition 0
                for h in range(2):
                    nc.tensor.matmul(rs[:, 128 * h:128 * h + 128],
                                     ones2[:, h:h + 1], pt,
                                     start=True, stop=True,
                                     tile_position=(0, 0))
                rcp = work.tile([1, 256], F32, name="rcp")
                nc.vector.reciprocal(rcp, rs)
                if phase == 2:
                    rcf2 = work.tile([128, 256], F32, name="rcf2")
                    nc.vector.memset(rcf2, 0.0)
                    nc.vector.tensor_copy(rcf2[0:1, :], rcp)
                else:
                    rcf2 = None
                # layered broadcast of 1/rowsum on Pool; rc folds into outproj
                scl = work.tile([128, 128], F32, name="scl")
                nc.gpsimd.partition_broadcast(scl, rcp[:, 128:256], channels=128)
                nc.gpsimd.partition_broadcast(
                    scl[0:64, :], rcp[:, 0:128], channels=64)

                # pass 2: G2^T = Z P~^T, masked
                g2m = work.tile([128, 256], ADT, name="g2m")
                for h in range(2):
                    s = slice(64 * h, 64 * h + 64)
                    nc.tensor.matmul(g2p[h], at[s, :], pt[s, :],
                                     start=True, stop=True,
                                     tile_position=(64 * h, 0))
                    nc.any.tensor_mul(g2m[:, 128 * h:128 * h + 128],
                                      g2p[h], triu2[:, 0:C])

                # attn^T = V^T G2m (+ T^T P~^T), then normalize via scl
                for h in range(2):
                    nc.tensor.matmul(
                        pan[64 * h:64 * h + 64, :],
                        vtok_sb[c][:, hp * 128 + 64 * h:hp * 128 + 64 * h + 64],
                        g2m[:, 128 * h:128 * h + 128],
                        start=True, stop=(c == 0),
                        tile_position=(0, 64 * h))
                if c > 0:
                    nc.tensor.matmul(pan, T_bd[hp], pt,
                                     start=False, stop=True,
                                     skip_group_check=True)
                nc.any.tensor_mul(attnT_sb[c][:, hp, :], pan, scl)

                # ---- state updates (block-diag accumulate in psum) ----
                Sp = state[:, 256 + 128 * hp:384 + 128 * hp]
                Tp = state[:, 128 * hp:128 * hp + 128]
                for h in range(2):
                    s = slice(64 * h, 64 * h + 64)
                    nc.tensor.matmul(Sp[s, s], ktc[:, s], z[:, s],
                                     start=False, stop=True,
                                     skip_group_check=True,
                                     tile_position=(0, 64 * h))
                    nc.tensor.matmul(
                        Tp[s, s], z[:, s],
                        vtok_sb[c][:, hp * 128 + 64 * h:hp * 128 + 64 * h + 64],
                        start=False, stop=True,
                        skip_group_check=True,
                        tile_position=(0, 64 * h))
                if c < NCH - 1 or phase == 2:
                    nc.any.tensor_copy(S_bd[hp], Sp)
                    nc.any.tensor_copy(T_bd[hp], Tp)
                if phase == 2:
                    Sc = work.tile([128, 128], F32, name="Sc")
                    nc.vector.tensor_copy(Sc, Sp)
                    Tc = work.tile([128, 128], F32, name="Tc")
                    nc.vector.tensor_copy(Tc, Tp)
                    return dict(z=z, gm=gm, pt=pt, rcf2=rcf2, scl=scl,
                                g2m=g2m, Sc=Sc, Tc=Tc)

            def out_block(c):
                tok = slice(c * C, (c + 1) * C)
                for oh in range(2):
                    po = psum.tile([128, 512], F32, tag="pp", bufs=2, name="po")
                    for et in range(2):
                        nc.tensor.matmul(
                            po, attnT_sb[c][:, et, :],
                            wo_sb[:, et, oh * 512:(oh + 1) * 512],
                            start=(et == 0), stop=(et == 1))
                    ob = obuf.tile([128, 512], F32, name="ob")
                    nc.any.tensor_scalar_mul(ob, po, rcc[:, c:c + 1])
                    nc.sync.dma_start(
                        out=out_d[tok, oh * 512:(oh + 1) * 512], in_=ob)

            if phase in (1, 2):
                proj_half(0)
                proj_half(1)
                proj_pq()
                for c in range(NCH):
                    proj_v(c)
            if phase == 1:
                # dump projections (bf16 bitcast to f32 cols)
                for et in range(2):
                    nc.sync.dma_start(out=out_d[et * 128:(et + 1) * 128, 0:512],
                                      in_=qTrc_sb[:, et, :].bitcast(F32))
                    nc.sync.dma_start(
                        out=out_d[256 + et * 128:256 + (et + 1) * 128, 0:512],
                        in_=kT_sb[:, et, :].bitcast(F32))
                    nc.sync.dma_start(
                        out=out_d[512 + et * 128:512 + (et + 1) * 128, 0:512],
                        in_=pcT_sb[:, et, :].bitcast(F32))
                    nc.sync.dma_start(
                        out=out_d[768 + et * 128:768 + (et + 1) * 128, 0:64],
                        in_=bdpq[:, et, :].bitcast(F32))
                nc.sync.dma_start(out=out_d[768:896, 128:256],
                                  in_=vtok_sb[0].bitcast(F32))

            if phase >= 3:
                proj_half(0)
                proj_pq()
                proj_v(0)
                filler = ([lambda tb=tb: proj_v(tb) for tb in range(1, 4)]
                          + [lambda et=et, k=k: proj_chain(k, et, 1)
                             for k in ("pc", "k", "q") for et in range(2)]
                          + [lambda tb=tb: proj_v(tb) for tb in range(4, NCH)])
                fi = 0
                for c in range(NCH):
                    for hp in range(2):
                        attn_call(c, hp)
                        # drip-feed remaining projection work between calls
                        for _ in range(2 if c < 4 else 0):
                            if fi < len(filler):
                                filler[fi]()
                                fi += 1
                    out_block(c)
                while fi < len(filler):
                    filler[fi]()
                    fi += 1
            elif phase == 2:
                # run chunks 0 and 1 for hp=0 only, dump intermediates
                t0 = attn_call(0, 0)
                t1 = attn_call(1, 0)
                for i, t in enumerate((t0, t1)):
                    base = i * 512
                    nc.sync.dma_start(out=out_d[base + 0:base + 128, 0:64],
                                      in_=t["z"].bitcast(F32))
                    nc.sync.dma_start(out=out_d[base + 0:base + 128, 64:192],
                                      in_=t["gm"].bitcast(F32))
                    nc.sync.dma_start(out=out_d[base + 0:base + 128, 192:256],
                                      in_=t["pt"].bitcast(F32))
                    nc.sync.dma_start(out=out_d[base + 128:base + 256, 0:256],
                                      in_=t["rcf2"])
                    nc.sync.dma_start(out=out_d[base + 128:base + 256, 256:512],
                                      in_=t["scl"])
                    nc.sync.dma_start(out=out_d[base + 256:base + 384, 0:128],
                                      in_=t["g2m"].bitcast(F32))
                    nc.sync.dma_start(out=out_d[base + 256:base + 384, 128:256],
                                      in_=t["Sc"])
                    nc.sync.dma_start(out=out_d[base + 256:base + 384, 256:384],
                                      in_=t["Tc"])
                    nc.sync.dma_start(out=out_d[base + 384:base + 512, 0:64],
                                      in_=attnT_sb[i][:, 0, :].bitcast(F32))

    # Patch the act-table map so Exp and Ln both resolve to the combined
    # natural_log_exp_and_others set (otherwise the load-placement pass
    # alternates exp_and_others <-> natural_log per chunk, ~42us of reloads).
    import concourse.bacc as _bacc_mod
    from concourse.hw_specs import get_activation_tables as _gat
    _orig_gat = _bacc_mod.get_activation_tables

    def _patched_gat(arch):
        t = _gat(arch)
        for name, s in t.items():
            if name != "natural_log_exp_and_others":
                s.discard(AF.Exp)
                s.discard(AF.Ln)
        return t

    _bacc_mod.get_activation_tables = _patched_gat
    try:
        nc.compile()
    finally:
        _bacc_mod.get_activation_tables = _orig_gat
    return nc


_CACHE = {}


def _get_nc():
    import os
    phase = int(os.environ.get("KPHASE", "3"))
    key = f"nc{phase}"
    if key not in _CACHE:
        _CACHE[key] = build_bass(phase)
    return _CACHE[key]


def make_in_maps(query, p, Wq, bq, Wpq, bpq, Wpc, bpc, Wk, bk, Wv, bv, Wo, bo):
    import ml_dtypes
    bf = ml_dtypes.bfloat16
    f32 = lambda a: np.ascontiguousarray(np.asarray(a), dtype=np.float32)
    query, p = f32(query), f32(p)
    Wq, Wpq, Wpc, Wk, Wv, Wo = map(f32, (Wq, Wpq, Wpc, Wk, Wv, Wo))
    bq, bpq, bpc, bk, bv, bo = map(f32, (bq, bpq, bpc, bk, bv, bo))
    rc = (1.0 / ((np.arange(N) + 1.0) * BETA)).astype(np.float32)
    rcb = np.ascontiguousarray(np.broadcast_to(rc[None, :], (128, N)))
    ones2 = np.zeros((128, 2), bf)
    ones2[0:64, 0] = 1
    ones2[64:128, 1] = 1
    rcc = np.ascontiguousarray(rc.reshape(NCH, 128).T)

    def col2(v):  # (256,) -> (128, 2)
        return np.ascontiguousarray(v.reshape(2, 128).T)

    in_maps = []
    for core in range(NCORES):
        b = core // 4
        hs = (core % 4) * HPC
        cols = slice(hs * DH, (hs + HPC) * DH)
        m = {
            "xT": np.ascontiguousarray(query[b].T).astype(bf),
            "pT": np.ascontiguousarray(p[b].T).astype(bf),
            "wq": np.ascontiguousarray((Wq[cols, :] * SCALE).T).astype(bf),
            "wk": np.ascontiguousarray(Wk[cols, :].T).astype(bf),
            "wv": np.ascontiguousarray(Wv[cols, :].T).astype(bf),
            "wpc": np.ascontiguousarray(Wpc[cols, :].T).astype(bf),
            "wpq": np.ascontiguousarray((Wpq[cols, :] * SCALE).T).astype(bf),
            "wo": np.ascontiguousarray(Wo[:, cols].T).astype(bf),
            "bq": col2(bq[cols] * SCALE),
            "bk": col2(bk[cols]),
            "bpc": col2(bpc[cols]),
            "bpq": col2(bpq[cols] * SCALE),
            "bvr": np.ascontiguousarray(bv[cols].reshape(1, E)).astype(bf),
            "rcb": rcb,
            "onesr": np.ones((1, 128), bf),
            "rcc": rcc,
            "ones2": ones2,
        }
        in_maps.append(m)
    return in_maps


def kernel(query, p, dec_input_mask=None, p_mask=None,
           Wq=None, bq=None, Wpq=None, bpq=None, Wpc=None, bpc=None,
           Wk=None, bk=None, Wv=None, bv=None, Wo=None, bo=None,
           _trace=False, _trace_kwargs=None):
    in_maps = make_in_maps(query, p, Wq, bq, Wpq, bpq, Wpc, bpc,
                           Wk, bk, Wv, bv, Wo, bo)
    res = run_bass_kernel_spmd(_get_nc(), in_maps, core_ids=list(range(NCORES)),
                               trace=_trace, **(_trace_kwargs or {}))
    bo = np.asarray(bo, dtype=np.float32)
    out = np.zeros((B, N, D), np.float32)
    out += bo.reshape(1, 1, D)
    for core in range(NCORES):
        out[core // 4] += res.results[core]["outp"]
    if _trace:
        kernel.last_result = res
    return out


# revision 42
# speedup vs baseline: 1.1430x; 1.0074x over previous
"""Trainium2 Bass kernel for nn_LunaCausalAttention.

Sharding: 8 cores; core c handles batch b = c//4 and heads hs = 4*(c%4) .. hs+4.

Restructured vs baseline:
- DMA ordered so the pc projection streams against the xt tiles (no startup
  bubble); per-projection psum chains get enough banks to pipeline.
- Pass-1 computed m-major (awT = Z^T tril(G) + S^T Q), with rc folded into a
  pre-scaled qTrc at projection time, so the softmax exp emerges directly in
  the [m, tok] layout pass-2 needs -- no P~ transposes on the critical path.
- Softmax normalization deferred: P~ left unnormalized; a per-(head, token)
  scale tile (built by tiny fp16 outer-product matmuls) is applied once when
  attn^T leaves psum.
- S/T state accumulated directly in a persistent psum bank by the PE
  (start=c==0), copied to sbuf bf16 once per chunk on the Act engine.
- Output projection bias moved to the host-side partial reduction.
"""
import numpy as np

import concourse.bass as bass
import concourse.mybir as mybir
import concourse.tile as tile
from concourse import bacc
from concourse.masks import make_upper_triangular, make_identity
from concourse.bass_utils import run_bass_kernel_spmd

# static shapes
B, N, D, M, H, DH = 2, 1024, 1024, 64, 16, 64
C = 128                 # token chunk
NCH = N // C            # 8 chunks
NCORES = 8
HPC = 4                 # heads per core
E = HPC * DH            # 256 per-core head features
NF = D // 128           # 8 contraction tiles
BETA = float(np.log(2.0))
SCALE = DH ** -0.5

F32 = mybir.dt.float32
F16 = mybir.dt.float16
BF16 = mybir.dt.bfloat16
ADT = BF16              # attention-core operand dtype
AF = mybir.ActivationFunctionType
ALU = mybir.AluOpType


def build_bass(phase=3):
    nc = bacc.Bacc(None, target_bir_lowering=False)

    # ---- I/O ----
    xT_d = nc.dram_tensor("xT", [D, N], BF16, kind="ExternalInput")       # query[b].T
    pT_d = nc.dram_tensor("pT", [D, M], BF16, kind="ExternalInput")       # p[b].T
    wq_d = nc.dram_tensor("wq", [D, E], BF16, kind="ExternalInput")       # scale folded
    wk_d = nc.dram_tensor("wk", [D, E], BF16, kind="ExternalInput")
    wv_d = nc.dram_tensor("wv", [D, E], BF16, kind="ExternalInput")
    wpc_d = nc.dram_tensor("wpc", [D, E], BF16, kind="ExternalInput")
    wpq_d = nc.dram_tensor("wpq", [D, E], BF16, kind="ExternalInput")     # scale folded
    wo_d = nc.dram_tensor("wo", [E, D], BF16, kind="ExternalInput")
    bq_d = nc.dram_tensor("bq", [128, 2], F32, kind="ExternalInput")      # [i,et]=b[128et+i]
    bk_d = nc.dram_tensor("bk", [128, 2], F32, kind="ExternalInput")
    bpc_d = nc.dram_tensor("bpc", [128, 2], F32, kind="ExternalInput")
    bpq_d = nc.dram_tensor("bpq", [128, 2], F32, kind="ExternalInput")
    bvr_d = nc.dram_tensor("bvr", [1, E], BF16, kind="ExternalInput")     # row form
    rcb_d = nc.dram_tensor("rcb", [128, N], F32, kind="ExternalInput")    # every row = rc
    ones_d = nc.dram_tensor("onesr", [1, 128], BF16, kind="ExternalInput")
    rcc_d = nc.dram_tensor("rcc", [128, NCH], F32, kind="ExternalInput")
    ones2_d = nc.dram_tensor("ones2", [128, 2], BF16, kind="ExternalInput")  # h indicator
    out_d = nc.dram_tensor("outp", [N, D], F32, kind="ExternalOutput")

    with tile.TileContext(nc) as tc:
        with (
            tc.tile_pool(name="singles", bufs=1) as singles,
            tc.tile_pool(name="work", bufs=5) as work,
            tc.tile_pool(name="obuf", bufs=4) as obuf,
            tc.tile_pool(name="psum", bufs=1, space="PSUM") as psum,
        ):
            # ---- constants (device-generated) ----
            triu2 = singles.tile([128, 2 * C], F32)     # two upper-tri copies
            make_upper_triangular(nc, triu2[:, 0:C], val=1.0, diag=True)
            make_upper_triangular(nc, triu2[:, C:2 * C], val=1.0, diag=True)
            identb = singles.tile([128, 128], ADT)
            make_identity(nc, identb)

            # ---- DMA, in compute-need order ----
            def load_w(name, dram):
                w = singles.tile([128, NF, E], BF16, name=name)
                nc.sync.dma_start(
                    out=w, in_=dram.rearrange("(f p) e -> p f e", p=128))
                return w

            def load_small(shape, dt, dram, name):
                t = singles.tile(shape, dt, name=name)
                nc.sync.dma_start(out=t, in_=dram[:, :])
                return t

            wpc_sb = load_w("wpc_sb", wpc_d)
            bpc_sb = load_small([128, 2], F32, bpc_d, "bpc_sb")
            xt_sb = []
            for f in range(NF):
                xt = singles.tile([128, N], BF16, name=f"xt{f}")
                nc.sync.dma_start(out=xt, in_=xT_d[f * 128:(f + 1) * 128, :])
                xt_sb.append(xt)
            wk_sb = load_w("wk_sb", wk_d)
            bk_sb = load_small([128, 2], F32, bk_d, "bk_sb")
            wq_sb = load_w("wq_sb", wq_d)
            bq_sb = load_small([128, 2], F32, bq_d, "bq_sb")
            rcb_sb = singles.tile([128, N], F32)
            nc.sync.dma_start(out=rcb_sb, in_=rcb_d[:, :])
            wpq_sb = load_w("wpq_sb", wpq_d)
            pT_sb = singles.tile([128, NF, M], BF16)
            nc.sync.dma_start(
                out=pT_sb, in_=pT_d.rearrange("(f p) m -> p f m", p=128))
            bpq_sb = load_small([128, 2], F32, bpq_d, "bpq_sb")
            wv_sb = load_w("wv_sb", wv_d)
            bvr_sb = load_small([1, E], BF16, bvr_d, "bvr_sb")
            ones = load_small([1, 128], BF16, ones_d, "ones")
            rcc = load_small([128, NCH], F32, rcc_d, "rcc")
            ones2 = load_small([128, 2], BF16, ones2_d, "ones2")
            wo_sb = singles.tile([128, 2, D], BF16)
            nc.sync.dma_start(
                out=wo_sb, in_=wo_d.rearrange("(t p) o -> p t o", p=128))

            # ---- persistent sbuf tiles ----
            pcT_sb = singles.tile([128, 2, N], ADT)     # [feat, hp, tok]
            kT_sb = singles.tile([128, 2, N], ADT)
            qTrc_sb = singles.tile([128, 2, N], ADT)    # q * rc(tok), bias folded
            bdpq = singles.tile([128, 2, 128], ADT)     # block-diag pq per hp
            nc.vector.memset(bdpq, 0.0)
            vtok_sb = [singles.tile([128, E], ADT, name=f"vtok{t}")
                       for t in range(NCH)]
            attnT_sb = [singles.tile([128, 2, C], ADT, name=f"attnT{t}")
                        for t in range(NCH)]
            S_bd = [singles.tile([128, 128], ADT, name=f"S{hp}") for hp in range(2)]
            T_bd = [singles.tile([128, 128], ADT, name=f"T{hp}") for hp in range(2)]

            # persistent psum state bank, all block-diagonal [128,128] regions:
            #   S (feat-pair x m-pair): [:, 0:128] hp0, [:, 128:256] hp1
            #   T (m-pair x feat-pair): [:, 256:384] hp0, [:, 384:512] hp1
            state = psum.tile([128, 512], F32, tag="state", name="state")
            nc.vector.memset(state, 0.0)

            # ---- projections, emitted per token-half (nh) so attention on
            # chunks 0-3 can overlap the nh=1 projection work ----
            def proj_chain(kind, et, nh):
                w_sb, b_sb, dst = {"pc": (wpc_sb, bpc_sb, pcT_sb),
                                   "k": (wk_sb, bk_sb, kT_sb),
                                   "q": (wq_sb, bq_sb, qTrc_sb)}[kind]
                pp = psum.tile([128, 512], F32, tag="pp", bufs=2, name="ppc")
                for f in range(NF):
                    nc.tensor.matmul(
                        pp, w_sb[:, f, et * 128:(et + 1) * 128],
                        xt_sb[f][:, nh * 512:(nh + 1) * 512],
                        start=(f == 0), stop=(f == NF - 1))
                if kind == "q":
                    nc.vector.scalar_tensor_tensor(
                        dst[:, et, nh * 512:(nh + 1) * 512], pp,
                        b_sb[:, et:et + 1],
                        rcb_sb[:, nh * 512:(nh + 1) * 512],
                        ALU.add, ALU.mult)
                else:
                    nc.scalar.activation(
                        dst[:, et, nh * 512:(nh + 1) * 512], pp,
                        AF.Identity, bias=b_sb[:, et:et + 1])

            def proj_half(nh):
                for kind in ("pc", "k", "q"):
                    for et in range(2):
                        proj_chain(kind, et, nh)

            def proj_pq():
                for hp in range(2):
                    ppq = psum.tile([128, 512], F32, tag="pp", bufs=2,
                                    name="pppq")
                    for f in range(NF):
                        nc.tensor.matmul(
                            ppq[:, 0:M], wpq_sb[:, f, hp * 128:(hp + 1) * 128],
                            pT_sb[:, f, :],
                            start=(f == 0), stop=(f == NF - 1))
                    for h in range(2):
                        sl = slice(64 * h, 64 * h + 64)
                        nc.vector.tensor_scalar_add(
                            bdpq[sl, hp, 64 * h:64 * h + 64], ppq[sl, 0:M],
                            bpq_sb[sl, hp:hp + 1])

            def proj_v(tb):
                pkv = psum.tile([128, 512], F32, tag="pp", bufs=2, name="pkv")
                for f in range(NF):
                    nc.tensor.matmul(
                        pkv[:, 0:E], xt_sb[f][:, tb * 128:(tb + 1) * 128],
                        wv_sb[:, f, :],
                        start=(f == 0), stop=False)
                nc.tensor.matmul(pkv[:, 0:E], ones, bvr_sb,
                                 start=False, stop=True)
                nc.any.tensor_copy(vtok_sb[tb], pkv[:, 0:E])

            # ---- attention ----
            def attn_call(c, hp):
                tok = slice(c * C, (c + 1) * C)
                # psum packing
                A = psum.tile([128, 512], F32, tag="pca", bufs=2, name="A")
                Bp = psum.tile([128, 512], F32, tag="pcb", bufs=2, name="Bp")
                Cp = psum.tile([128, 512], F32, tag="pcc", bufs=1, name="Cp")
                pz = A[:, 0:128]
                awT = A[:, 128:256]
                gmp = (A[:, 256:384], Bp[:, 0:128])
                g2p = (A[:, 384:512], Bp[:, 128:256])
                pan = Bp[:, 256:384]
                pkt = Cp[:, 0:64].bitcast(ADT)
                att = Cp[:, 64:128].bitcast(ADT)
                rs = Cp[0:1, 128:384]

                # Z_c: pz = pcT^T @ bdpq  -> [tok, m-pair]
                nc.tensor.matmul(pz, pcT_sb[:, hp, tok], bdpq[:, hp, :],
                                 start=True, stop=True)
                ez = work.tile([128, 128], F32, name="ez")
                nc.scalar.activation(ez, pz, AF.Exp, scale=BETA)
                z = work.tile([128, 128], ADT, name="z")
                nc.scalar.activation(z, ez, AF.Ln, bias=1.0, scale=1.0)

                # K_tok via PE transpose
                ktc = work.tile([128, 128], ADT, name="ktc")
                nc.tensor.transpose(pkt, kT_sb[:, hp, tok], identb)
                nc.any.tensor_copy(ktc, pkt)

                # Z^T via PE transpose
                at = work.tile([128, 128], ADT, name="at")
                nc.tensor.transpose(att, z, identb)
                nc.any.tensor_copy(at, att)

                # G^T = K Q_rc^T (rc folded in qTrc), masked
                gm = work.tile([128, 256], ADT, name="gm")
                for h in range(2):
                    s = slice(64 * h, 64 * h + 64)
                    nc.tensor.matmul(gmp[h], kT_sb[s, hp, tok],
                                     qTrc_sb[s, hp, tok], start=True, stop=True,
                                     tile_position=(64 * h, 0))
                    nc.any.tensor_mul(gm[:, 128 * h:128 * h + 128],
                                      gmp[h], triu2[:, 0:C])

                # awT[m-pair, tok] = Z^T Gm (+ S^T Q_rc)
                for h in range(2):
                    s = slice(64 * h, 64 * h + 64)
                    nc.tensor.matmul(awT[s, :], z[:, s],
                                     gm[:, 128 * h:128 * h + 128],
                                     start=True, stop=(c == 0),
                                     tile_position=(0, 64 * h))
                if c > 0:
                    nc.tensor.matmul(awT, S_bd[hp], qTrc_sb[:, hp, tok],
                                     start=False, stop=True,
                                     skip_group_check=True)

                # P~^T = exp(awT), unnormalized, directly m-major
                pt = work.tile([128, 128], ADT, name="pt")
                nc.scalar.activation(pt, awT, AF.Exp, scale=1.0)

                # rowsums over m (partition dim) -> [1, 2*128] on partition 0
                for h in range(2):
                    nc.tensor.matmul(rs[:, 128 * h:128 * h + 128],
                                     ones2[:, h:h + 1], pt,
                                     start=True, stop=True,
                                     tile_position=(0, 0))
                rcp = work.tile([1, 256], F32, name="rcp")
                nc.vector.reciprocal(rcp, rs)
                if phase == 2:
                    rcf2 = work.tile([128, 256], F32, name="rcf2")
                    nc.vector.memset(rcf2, 0.0)
                    nc.vector.tensor_copy(rcf2[0:1, :], rcp)
                else:
                    rcf2 = None
                # layered broadcast of 1/rowsum on Pool; rc folds into outproj
                scl = work.tile([128, 128], F32, name="scl")
                nc.gpsimd.partition_broadcast(scl, rcp[:, 128:256], channels=128)
                nc.gpsimd.partition_broadcast(
                    scl[0:64, :], rcp[:, 0:128], channels=64)

                # pass 2: G2^T = Z P~^T, masked
                g2m = work.tile([128, 256], ADT, name="g2m")
                for h in range(2):
                    s = slice(64 * h, 64 * h + 64)
                    nc.tensor.matmul(g2p[h], at[s, :], pt[s, :],
                                     start=True, stop=True,
                                     tile_position=(64 * h, 0))
                    nc.any.tensor_mul(g2m[:, 128 * h:128 * h + 128],
                                      g2p[h], triu2[:, 0:C])

                # attn^T = V^T G2m (+ T^T P~^T), then normalize via scl
                for h in range(2):
                    nc.tensor.matmul(
                        pan[64 * h:64 * h + 64, :],
                        vtok_sb[c][:, hp * 128 + 64 * h:hp * 128 + 64 * h + 64],
                        g2m[:, 128 * h:128 * h + 128],
                        start=True, stop=(c == 0),
                        tile_position=(0, 64 * h))
                if c > 0:
                    nc.tensor.matmul(pan, T_bd[hp], pt,
                                     start=False, stop=True,
                                     skip_group_check=True)
                nc.any.tensor_mul(attnT_sb[c][:, hp, :], pan, scl)

                # ---- state updates (block-diag accumulate in psum) ----
                Sp = state[:, 256 + 128 * hp:384 + 128 * hp]
                Tp = state[:, 128 * hp:128 * hp + 128]
                for h in range(2):
                    s = slice(64 * h, 64 * h + 64)
                    nc.tensor.matmul(Sp[s, s], ktc[:, s], z[:, s],
                                     start=False, stop=True,
                                     skip_group_check=True,
                                     tile_position=(0, 64 * h))
                    nc.tensor.matmul(
                        Tp[s, s], z[:, s],
                        vtok_sb[c][:, hp * 128 + 64 * h:hp * 128 + 64 * h + 64],
                        start=False, stop=True,
                        skip_group_check=True,
                        tile_position=(0, 64 * h))
                if c < NCH - 1 or phase == 2:
                    nc.any.tensor_copy(S_bd[hp], Sp)
                    nc.any.tensor_copy(T_bd[hp], Tp)
                if phase == 2:
                    Sc = work.tile([128, 128], F32, name="Sc")
                    nc.vector.tensor_copy(Sc, Sp)
                    Tc = work.tile([128, 128], F32, name="Tc")
                    nc.vector.tensor_copy(Tc, Tp)
                    return dict(z=z, gm=gm, pt=pt, rcf2=rcf2, scl=scl,
                                g2m=g2m, Sc=Sc, Tc=Tc)

            def out_block(c):
                tok = slice(c * C, (c + 1) * C)
                for oh in range(2):
                    po = psum.tile([128, 512], F32, tag="pp", bufs=2, name="po")
                    for et in range(2):
                        nc.tensor.matmul(
                            po, attnT_sb[c][:, et, :],
                            wo_sb[:, et, oh * 512:(oh + 1) * 512],
                            start=(et == 0), stop=(et == 1))
                    ob = obuf.tile([128, 512], F32, name="ob")
                    nc.any.tensor_scalar_mul(ob, po, rcc[:, c:c + 1])
                    nc.sync.dma_start(
                        out=out_d[tok, oh * 512:(oh + 1) * 512], in_=ob)

            if phase in (1, 2):
                proj_half(0)
                proj_half(1)
                proj_pq()
                for c in range(NCH):
                    proj_v(c)
            if phase == 1:
                # dump projections (bf16 bitcast to f32 cols)
                for et in range(2):
                    nc.sync.dma_start(out=out_d[et * 128:(et + 1) * 128, 0:512],
                                      in_=qTrc_sb[:, et, :].bitcast(F32))
                    nc.sync.dma_start(
                        out=out_d[256 + et * 128:256 + (et + 1) * 128, 0:512],
                        in_=kT_sb[:, et, :].bitcast(F32))
                    nc.sync.dma_start(
                        out=out_d[512 + et * 128:512 + (et + 1) * 128, 0:512],
                        in_=pcT_sb[:, et, :].bitcast(F32))
                    nc.sync.dma_start(
                        out=out_d[768 + et * 128:768 + (et + 1) * 128, 0:64],
                        in_=bdpq[:, et, :].bitcast(F32))
                nc.sync.dma_start(out=out_d[768:896, 128:256],
                                  in_=vtok_sb[0].bitcast(F32))

            if phase >= 3:
                proj_half(0)
                proj_pq()
                proj_v(0)
                filler = ([lambda tb=tb: proj_v(tb) for tb in range(1, 4)]
                          + [lambda et=et, k=k: proj_chain(k, et, 1)
                             for k in ("pc", "k", "q") for et in range(2)]
                          + [lambda tb=tb: proj_v(tb) for tb in range(4, NCH)])
                fi = 0
                for c in range(NCH):
                    for hp in range(2):
                        attn_call(c, hp)
                        # drip-feed remaining projection work between calls
                        for _ in range(1 if c < 6 else 0):
                            if fi < len(filler):
                                filler[fi]()
                                fi += 1
                    out_block(c)
                while fi < len(filler):
                    filler[fi]()
                    fi += 1
            elif phase == 2:
                # run chunks 0 and 1 for hp=0 only, dump intermediates
                t0 = attn_call(0, 0)
                t1 = attn_call(1, 0)
                for i, t in enumerate((t0, t1)):
                    base = i * 512
                    nc.sync.dma_start(out=out_d[base + 0:base + 128, 0:64],
                                      in_=t["z"].bitcast(F32))
                    nc.sync.dma_start(out=out_d[base + 0:base + 128, 64:192],
                                      in_=t["gm"].bitcast(F32))
                    nc.sync.dma_start(out=out_d[base + 0:base + 128, 192:256],
                                      in_=t["pt"].bitcast(F32))
                    nc.sync.dma_start(out=out_d[base + 128:base + 256, 0:256],
                                      in_=t["rcf2"])
                    nc.sync.dma_start(out=out_d[base + 128:base + 256, 256:512],
                                      in_=t["scl"])
                    nc.sync.dma_start(out=out_d[base + 256:base + 384, 0:128],
                                      in_=t["g2m"].bitcast(F32))
                    nc.sync.dma_start(out=out_d[base + 256:base + 384, 128:256],
                                      in_=t["Sc"])
                    nc.sync.dma_start(out=out_d[base + 256:base + 384, 256:384],
                                      in_=t["Tc"])
                    nc.sync.dma_start(out=out_d[base + 384:base + 512, 0:64],
                                      in_=attnT_sb[i][:, 0, :].bitcast(F32))

    # Patch the act-table map so Exp and Ln both resolve to the combined
    # natural_log_exp_and_others set (otherwise the load-placement pass
    # alternates exp_and_others <-> natural_log per chunk, ~42us of reloads).
    import concourse.bacc as _bacc_mod
    from concourse.hw_specs import get_activation_tables as _gat
    _orig_gat = _bacc_mod.get_activation_tables

    def _patched_gat(arch):
        t = _gat(arch)
        for name, s in t.items():
            if name != "natural_log_exp_and_others":
                s.discard(AF.Exp)
                s.discard(AF.Ln)
        return t

    _bacc_mod.get_activation_tables = _patched_gat
    try:
        nc.compile()
    finally:
        _bacc_mod.get_activation_tables = _orig_gat
    return nc


_CACHE = {}


def _get_nc():
    import os
    phase = int(os.environ.get("KPHASE", "3"))
    key = f"nc{phase}"
    if key not in _CACHE:
        _CACHE[key] = build_bass(phase)
    return _CACHE[key]


def make_in_maps(query, p, Wq, bq, Wpq, bpq, Wpc, bpc, Wk, bk, Wv, bv, Wo, bo):
    import ml_dtypes
    bf = ml_dtypes.bfloat16
    f32 = lambda a: np.ascontiguousarray(np.asarray(a), dtype=np.float32)
    query, p = f32(query), f32(p)
    Wq, Wpq, Wpc, Wk, Wv, Wo = map(f32, (Wq, Wpq, Wpc, Wk, Wv, Wo))
    bq, bpq, bpc, bk, bv, bo = map(f32, (bq, bpq, bpc, bk, bv, bo))
    rc = (1.0 / ((np.arange(N) + 1.0) * BETA)).astype(np.float32)
    rcb = np.ascontiguousarray(np.broadcast_to(rc[None, :], (128, N)))
    ones2 = np.zeros((128, 2), bf)
    ones2[0:64, 0] = 1
    ones2[64:128, 1] = 1
    rcc = np.ascontiguousarray(rc.reshape(NCH, 128).T)

    def col2(v):  # (256,) -> (128, 2)
        return np.ascontiguousarray(v.reshape(2, 128).T)

    in_maps = []
    for core in range(NCORES):
        b = core // 4
        hs = (core % 4) * HPC
        cols = slice(hs * DH, (hs + HPC) * DH)
        m = {
            "xT": np.ascontiguousarray(query[b].T).astype(bf),
            "pT": np.ascontiguousarray(p[b].T).astype(bf),
            "wq": np.ascontiguousarray((Wq[cols, :] * SCALE).T).astype(bf),
            "wk": np.ascontiguousarray(Wk[cols, :].T).astype(bf),
            "wv": np.ascontiguousarray(Wv[cols, :].T).astype(bf),
            "wpc": np.ascontiguousarray(Wpc[cols, :].T).astype(bf),
            "wpq": np.ascontiguousarray((Wpq[cols, :] * SCALE).T).astype(bf),
            "wo": np.ascontiguousarray(Wo[:, cols].T).astype(bf),
            "bq": col2(bq[cols] * SCALE),
            "bk": col2(bk[cols]),
            "bpc": col2(bpc[cols]),
            "bpq": col2(bpq[cols] * SCALE),
            "bvr": np.ascontiguousarray(bv[cols].reshape(1, E)).astype(bf),
            "rcb": rcb,
            "onesr": np.ones((1, 128), bf),
            "rcc": rcc,
            "ones2": ones2,
        }
        in_maps.append(m)
    return in_maps


def kernel(query, p, dec_input_mask=None, p_mask=None,
           Wq=None, bq=None, Wpq=None, bpq=None, Wpc=None, bpc=None,
           Wk=None, bk=None, Wv=None, bv=None, Wo=None, bo=None,
           _trace=False, _trace_kwargs=None):
    in_maps = make_in_maps(query, p, Wq, bq, Wpq, bpq, Wpc, bpc,
                           Wk, bk, Wv, bv, Wo, bo)
    res = run_bass_kernel_spmd(_get_nc(), in_maps, core_ids=list(range(NCORES)),
                               trace=_trace, **(_trace_kwargs or {}))
    bo = np.asarray(bo, dtype=np.float32)
    out = np.zeros((B, N, D), np.float32)
    out += bo.reshape(1, 1, D)
    for core in range(NCORES):
        out[core // 4] += res.results[core]["outp"]
    if _trace:
        kernel.last_result = res
    return out
